# revision 1
# baseline (speedup 1.0000x reference)
"""Trainium2 Bass kernel for nn_Attention_39573828665647.

GQA causal attention block (B=4, S=1024, DIM=2048, 32 q heads / 8 kv heads,
hd=64) with RoPE, sharded over 8 NeuronCores as (batch x head-half):
core = 2*b + hh handles batch b and kv groups [4hh, 4hh+4) (16 q heads).
Each core computes a partial output projection over its 1024 o-dims; the
host sums the two partials per batch.

On-device pipeline (all matmuls in fp32r = TF32):
  A: qkT = wqkv_slice @ x^T in transposed layout [heads*hd, s]; RoPE fused
     via stream_shuffle + 2 mul + add (interleaved pair rotation), q
     pre-scaled by 1/sqrt(hd) through the host-built cos/sin tables.
  B: v = x @ wv^T in natural layout [s, hd], augmented with a ones column
     so the attention matmul also produces softmax denominators.
  C: per head: S^T[sk,sq] = k^T.T @ q^T (PE), exp on ACT (no max
     subtraction - scores are O(5) bounded), causal handled by ragged
     tiles + a gpsimd affine_select on the diagonal chunk,
     o_aug[65,sq] = [v|1]^T @ expT accumulated over sk tiles.
  D: per-column normalization: reciprocal of the denominator row,
     partition-broadcast via a DRAM-bounce DMA.
  E: out[s,o] = o^T.T @ wo^T (wo prefetched during C), psum -> sbuf -> DRAM.
"""

import numpy as np

B, S, DIM = 4, 1024, 2048
NH, NKV, HD = 32, 8, 64
P = 128
ND = DIM // P  # 16 d-tiles

_SWAP_ADJ = [i ^ 1 for i in range(32)]  # pairwise partition swap within quadrants

_CACHE = {}


def host_prep(x, freqs_cos, freqs_sin, wqkv, wo):
    """Build the 8 per-core input dicts."""
    x = np.ascontiguousarray(np.asarray(x, np.float32))
    wqkv = np.asarray(wqkv, np.float32)
    wo = np.asarray(wo, np.float32)
    cos = np.asarray(freqs_cos, np.float32)
    sin = np.asarray(freqs_sin, np.float32)

    cosT, sinT = cos.T, sin.T                      # [32, S]
    C64 = np.repeat(cosT, 2, axis=0)               # [64, S]
    Ss64 = np.repeat(sinT, 2, axis=0).copy()
    Ss64[0::2] *= -1.0                             # even rows -sin, odd +sin
    C64 = np.ascontiguousarray(C64, dtype=np.float32)
    Ss64 = np.ascontiguousarray(Ss64, dtype=np.float32)
    scale = np.float32(1.0 / np.sqrt(HD))
    Cq, Sq = C64 * scale, Ss64 * scale      # [64, S]; kernel duplicates rows
    Ck, Sk = C64, Ss64
    mask = np.triu(np.ones((P, P), np.float32))    # 1 where sq >= sk

    woT_full = np.ascontiguousarray(wo.T)          # [d', o]
    xT_full = np.ascontiguousarray(x.transpose(0, 2, 1))  # [B, DIM, S]
    wqkvT_full = np.ascontiguousarray(wqkv.T)      # [DIM, 3072]
    in_maps = []
    for core in range(8):
        b, hh = core // 2, core % 2
        groups = range(4 * hh, 4 * hh + 4)
        qheads = range(16 * hh, 16 * hh + 16)
        # assemble wqkvT from contiguous 64-column blocks (memcpy-speed)
        wqkvT = np.empty((DIM, 1536), np.float32)
        col = 0
        blocks = ([(h // 4 * 6 + h % 4) * 64 for h in qheads]
                  + [(g * 6 + 4) * 64 for g in groups]
                  + [(g * 6 + 5) * 64 for g in groups])
        for c0 in blocks:
            wqkvT[:, col:col + 64] = wqkvT_full[:, c0:c0 + 64]
            col += 64
        in_maps.append({
            "xT": xT_full[b],                                      # [2048, 1024]
            "wqkvT": wqkvT,                                        # [2048, 1536]
            "woT": np.ascontiguousarray(woT_full[1024 * hh:1024 * hh + 1024]),
            "Cq": Cq, "Sq": Sq, "Ck": Ck, "Sk": Sk, "mask": mask,
        })
    return in_maps


def build_nc(reps=1):
    from contextlib import ExitStack
    import concourse.bacc as bacc
    import concourse.bass as bass
    import concourse.tile as tile
    import concourse.mybir as mybir

    f32 = mybir.dt.float32
    f32r = mybir.dt.float32r
    EXP = mybir.ActivationFunctionType.Exp

    nc = bacc.Bacc("TRN2", target_bir_lowering=False, debug=False)
    xT_d = nc.dram_tensor("xT", [DIM, S], f32r, kind="ExternalInput")
    wqkvT_d = nc.dram_tensor("wqkvT", [DIM, 1536], f32r, kind="ExternalInput")
    woT_d = nc.dram_tensor("woT", [1024, DIM], f32r, kind="ExternalInput")
    Cq_d = nc.dram_tensor("Cq", [64, S], f32, kind="ExternalInput")
    Sq_d = nc.dram_tensor("Sq", [64, S], f32, kind="ExternalInput")
    Ck_d = nc.dram_tensor("Ck", [64, S], f32, kind="ExternalInput")
    Sk_d = nc.dram_tensor("Sk", [64, S], f32, kind="ExternalInput")
    out_d = nc.dram_tensor("out", [S, DIM], f32, kind="ExternalOutput")

    def emit(tc, pfx):
        with ExitStack() as stack:
            resid = stack.enter_context(tc.tile_pool(name=pfx + "resid", bufs=1))

            def rtile(shape, dt_, nm):
                return resid.tile(shape, dt_, tag=pfx + nm, name=pfx + nm)

            q_sb = [rtile([P, S], f32r, f"q{i}") for i in range(8)]
            k_sb = [rtile([P, S], f32r, f"k{g}") for g in range(4)]
            vaug = [rtile([P, 4, 65], f32r, f"va{i}") for i in range(8)]

            # ------------- Stage A + B: projections + rope -------------
            with tc.tile_pool(name=pfx + "xres", bufs=1) as xres_pool, \
                 tc.tile_pool(name=pfx + "ropeconst", bufs=1) as rc_pool, \
                 tc.tile_pool(name=pfx + "wstream", bufs=1) as w_pool, \
                 tc.tile_pool(name=pfx + "ropetmp", bufs=3) as rt_pool, \
                 tc.tile_pool(name=pfx + "psumA", bufs=8, space="PSUM") as psA:

                # PE warmup: spin matmuls on const data while the first
                # DMAs land, so HAM un-throttles and PE isn't idle. Uses a
                # psA slot so nothing downstream waits on a pool release.
                wmt = rt_pool.tile([P, P], f32r, tag="sh", name=pfx + "wm")
                nc.vector.tensor_copy(wmt[:], nc.const_aps.tensor(0.0, (P, P), f32))
                wps = psA.tile([P, P], f32, tag="acc", name=pfx + "wps")
                for _ in range(26):
                    nc.tensor.matmul(wps[:], wmt[:], wmt[:], start=True, stop=True)

                xres, wq = [], []
                c_sb = {}
                for d in range(ND):
                    xt = xres_pool.tile([P, S], f32r, tag=f"x{d}",
                                        name=pfx + f"x{d}")
                    wt = w_pool.tile([P, 512], f32r, tag=f"wq{d}",
                                     name=pfx + f"wq{d}")
                    eng_a = nc.sync if d % 2 == 0 else nc.scalar
                    eng_b = nc.scalar if d % 2 == 0 else nc.sync
                    # halves on opposite queues: t=0 matmuls start sooner
                    eng_a.dma_start(out=xt[:, 0:512],
                                    in_=xT_d[d * P:(d + 1) * P, 0:512])
                    eng_b.dma_start(out=wt[:],
                                    in_=wqkvT_d[d * P:(d + 1) * P, 0:512])
                    eng_a.dma_start(out=xt[:, 512:S],
                                    in_=xT_d[d * P:(d + 1) * P, 512:S])
                    xres.append(xt)
                    wq.append(wt)
                    if d == 10:  # rope tables (rows 64-127 duplicate 0-63:
                        # DMA half, duplicate on the idle ACT engine - halves
                        # the tables' claim on the DMA-fill window)
                        for i, (nm, dr) in enumerate(
                                (("Cq", Cq_d), ("Sq", Sq_d),
                                 ("Ck", Ck_d), ("Sk", Sk_d))):
                            ct = rc_pool.tile([P, S], f32, tag=nm, name=pfx + nm)
                            (nc.sync if i % 2 else nc.scalar).dma_start(
                                out=ct[0:64, :], in_=dr[:])
                            nc.scalar.copy(ct[64:128, :], ct[0:64, :])
                            c_sb[nm] = ct

                def rope(ptile, at, t):
                    sl = slice(t * 512, t * 512 + 512)
                    is_q = at < 8
                    C_ = c_sb["Cq" if is_q else "Ck"]
                    S_ = c_sb["Sq" if is_q else "Sk"]
                    sh = rt_pool.tile([P, 512], f32, tag="sh",
                                      name=pfx + f"sh{at}_{t}")
                    m1 = rt_pool.tile([P, 512], f32, tag="m1",
                                      name=pfx + f"m1_{at}_{t}")
                    m2 = rt_pool.tile([P, 512], f32, tag="m2",
                                      name=pfx + f"m2_{at}_{t}")
                    nc.vector.stream_shuffle(sh[:], ptile[:], _SWAP_ADJ)
                    nc.vector.tensor_mul(m1[:], ptile[:], C_[:, sl])
                    nc.gpsimd.tensor_mul(m2[:], sh[:], S_[:, sl])
                    if is_q:
                        nc.gpsimd.tensor_add(q_sb[at][:, sl], m1[:], m2[:])
                    else:
                        ro = rt_pool.tile([P, 512], f32r, tag="ro",
                                          name=pfx + f"ro{at}_{t}")
                        nc.gpsimd.tensor_add(ro[:], m1[:], m2[:])
                        for half in (0, 1):
                            g = 2 * (at - 8) + half
                            src = ro[half * 64:half * 64 + 64, :]
                            nc.scalar.copy(k_sb[g][0:64, sl], src)
                            nc.scalar.copy(k_sb[g][64:128, sl], src)

                def qk_group(ats, wtiles, coff):
                    for t in (0, 1):
                        for at in ats:
                            pt = psA.tile([P, 512], f32, tag="acc",
                                          name=pfx + f"acc{at}_{t}")
                            for d in range(ND):
                                nc.tensor.matmul(
                                    pt[:],
                                    wtiles[d][:, (at - coff) * P:(at - coff + 1) * P],
                                    xres[d][:, t * 512:(t + 1) * 512],
                                    start=(d == 0), stop=(d == ND - 1))
                            rope(pt, at, t)

                qk_group(range(4), wq, 0)

                wq2 = []
                for d in range(ND):
                    wt = w_pool.tile([P, 512], f32r, tag=f"wq{d}",
                                     name=pfx + f"w2_{d}")
                    (nc.sync if d % 2 else nc.scalar).dma_start(
                        out=wt[:], in_=wqkvT_d[d * P:(d + 1) * P, 512:1024])
                    wq2.append(wt)
                qk_group(range(4, 8), wq2, 4)

                wkv = []
                for d in range(ND):
                    wt = w_pool.tile([P, 512], f32r, tag=f"wq{d}",
                                     name=pfx + f"w3_{d}")
                    (nc.sync if d % 2 else nc.scalar).dma_start(
                        out=wt[:], in_=wqkvT_d[d * P:(d + 1) * P, 1024:1536])
                    wkv.append(wt)
                qk_group((8, 9), wkv, 8)

                # Stage B: v projection (natural layout) + ones augmentation
                for st in range(8):
                    pt = psA.tile([P, 256], f32, tag="acc", name=pfx + f"vacc{st}")
                    for d in range(ND):
                        nc.tensor.matmul(
                            pt[:], xres[d][:, st * P:(st + 1) * P],
                            wkv[d][:, 256:512], start=(d == 0), stop=(d == ND - 1))
                    nc.vector.tensor_copy(vaug[st][:, :, 64],
                                          nc.const_aps.tensor(1.0, (P, 4), f32))
                    for g in range(4):
                        nc.scalar.copy(
                            vaug[st][:, g, 0:64], pt[:, g * 64:(g + 1) * 64])

            # ------------- Stage C + D: attention -------------
            o_pool = stack.enter_context(tc.tile_pool(name=pfx + "opool", bufs=1))
            o_sb = [o_pool.tile([P, S], f32r, tag=f"o{i}", name=pfx + f"o{i}")
                    for i in range(8)]
            wo_pool = stack.enter_context(tc.tile_pool(name=pfx + "wo", bufs=1))
            wo_sb = {}

            def load_wo(i):
                ot, dt_ = i // 8, i % 8
                w = wo_pool.tile([P, 512], f32r, tag=f"wo{ot}_{dt_}",
                                 name=pfx + f"wo{ot}_{dt_}")
                nc.scalar.dma_start(
                    out=w[:],
                    in_=woT_d[dt_ * P:(dt_ + 1) * P, ot * 512:(ot + 1) * 512])
                wo_sb[(ot, dt_)] = w

            with tc.tile_pool(name=pfx + "expT", bufs=6) as e_pool, \
                 tc.tile_pool(name=pfx + "rdram", bufs=4, space="DRAM") as rd_pool, \
                 tc.tile_pool(name=pfx + "normtmp", bufs=4) as n_pool, \
                 tc.tile_pool(name=pfx + "outsb", bufs=4) as ob_pool, \
                 tc.tile_pool(name=pfx + "psumS", bufs=2, space="PSUM") as psS, \
                 tc.tile_pool(name=pfx + "psumO", bufs=4, space="PSUM") as psO:

                def normalize(h, t, opsum):
                    par = h % 2
                    r = n_pool.tile([1, 512], f32, tag="r", name=pfx + f"r{h}_{t}")
                    nc.vector.reciprocal(r[:], opsum[64:65, :])
                    rd = rd_pool.tile([1, 512], f32, tag="rd",
                                      name=pfx + f"rd{h}_{t}")
                    nc.sync.dma_start(out=rd[:], in_=r[:])
                    rb = n_pool.tile([64, 512], f32, tag="rb",
                                     name=pfx + f"rb{h}_{t}")
                    rdap = rd[:]
                    bcast = bass.AP(tensor=rdap.tensor, offset=rdap.offset,
                                    ap=[[0, 64]] + [list(p) for p in rdap.ap[1:]])
                    nc.sync.dma_start(out=rb[:], in_=bcast)
                    dst = o_sb[h // 2][par * 64:par * 64 + 64,
                                       t * 512:(t + 1) * 512]
                    nc.vector.tensor_mul(dst, opsum[0:64, :], rb[:])

                for h in range(16):
                    g = h // 4
                    par = h % 2
                    qh = q_sb[h // 2][par * 64:par * 64 + 64, :]
                    kh = k_sb[g][par * 64:par * 64 + 64, :]
                    opsum = [psO.tile([65, 512], f32, tag="op",
                                      name=pfx + f"op{h}_{t}") for t in (0, 1)]
                    for j in range(8):
                        lo = j * P
                        spsum = psS.tile([P, S], f32, tag="sp",
                                         name=pfx + f"sp{h}_{j}")
                        if j < 4:  # first sq-half (ragged)
                            nc.tensor.matmul(
                                spsum[:, lo:512], kh[:, j * P:(j + 1) * P],
                                qh[:, lo:512], start=True, stop=True)
                        hi = max(lo, 512)  # second half ragged too: only
                        nc.tensor.matmul(  # sq >= sk columns are needed
                            spsum[:, hi:S], kh[:, j * P:(j + 1) * P],
                            qh[:, hi:S], start=True, stop=True)
                        et = e_pool.tile([P, S], f32r, tag="et",
                                         name=pfx + f"et{h}_{j}")
                        nc.scalar.activation(et[:, lo:S], spsum[:, lo:S], EXP)
                        # diagonal chunk: zero sq < sk
                        nc.gpsimd.affine_select(
                            out=et[:, lo:lo + P], in_=et[:, lo:lo + P],
                            pattern=[[1, P]], channel_multiplier=-1,
                            base=0, compare_op=mybir.AluOpType.is_ge, fill=0.0)
                        if j < 4:
                            nc.tensor.matmul(
                                opsum[0][0:65, lo:512], vaug[j][:, g, :],
                                et[:, lo:512], start=(j == 0), stop=(j == 3))
                        lo1 = max(lo - 512, 0)
                        nc.tensor.matmul(
                            opsum[1][0:65, lo1:512], vaug[j][:, g, :],
                            et[:, 512 + lo1:S], start=(j == 0), stop=(j == 7))
                        if j == 3:
                            normalize(h, 0, opsum[0])
                    normalize(h, 1, opsum[1])
                    load_wo(2 * h)
                    load_wo(2 * h + 1)

                # ------------- Stage E: output projection -------------
                # pe tiles share psO's "op" slots: stage E acquires banks at
                # slot granularity as the last heads' opsums retire (no
                # pool-level barrier between attention and the projection).
                for ot in range(4):
                    for sc in range(8):
                        pe = psO.tile([P, 512], f32, tag="op",
                                      name=pfx + f"pe{ot}_{sc}")
                        for dt_ in range(8):
                            nc.tensor.matmul(
                                pe[:], o_sb[dt_][:, sc * P:(sc + 1) * P],
                                wo_sb[(ot, dt_)][:],
                                start=(dt_ == 0), stop=(dt_ == 7))
                        ob = ob_pool.tile([P, 512], f32, tag="ob",
                                          name=pfx + f"ob{ot}_{sc}")
                        nc.vector.tensor_copy(ob[:], pe[:])
                        (nc.sync if sc % 2 else nc.scalar).dma_start(
                            out=out_d[sc * P:(sc + 1) * P,
                                      ot * 512:(ot + 1) * 512],
                            in_=ob[:])

    with tile.TileContext(nc) as tc:
        for rep in range(reps):
            emit(tc, f"r{rep}_" if reps > 1 else "")

    nc.compile()
    return nc


def _get_nc():
    if "nc" not in _CACHE:
        _CACHE["nc"] = build_nc()
    return _CACHE["nc"]


def kernel(**inputs):
    from concourse.bass_utils import run_bass_kernel_spmd
    nc = _get_nc()
    in_maps = host_prep(**inputs)
    res = run_bass_kernel_spmd(nc, in_maps, core_ids=list(range(8)))
    outs = [res.results[c]["out"] for c in range(8)]
    full = np.stack([outs[2 * b] + outs[2 * b + 1] for b in range(B)])
    return full.astype(np.float32)


if __name__ == "__main__":
    nc = build_nc()
    print("build ok")



# revision 29
# speedup vs baseline: 2.0752x; 2.0752x over previous
"""Trainium2 Bass kernel for nn_Attention_39573828665647.

GQA causal attention block (B=4, S=1024, DIM=2048, 32 q heads / 8 kv heads,
hd=64) with RoPE, sharded over 8 NeuronCores as (batch x head-half):
core = 2*b + hh handles batch b and kv groups [4hh, 4hh+4) (16 q heads).
Each core computes a partial output projection over its 1024 o-dims; the
host sums the two partials per batch.

v2 design (bf16 compute, software-pipelined emission):
  All matmul operands are bf16 (psum accumulation fp32): full PE rate at
  any free size, half the DMA bytes and SBUF of fp32.
  Phase 1: k projection (d-outer accumulation so it starts as soon as the
    first x/w d-tiles land), RoPE on k, then v projection + ones-augmented
    vaug tiles.
  Phase 2: eight blocks; block qt projects q-tile qt (2 heads) with its
    32 matmuls EMITTED AS PE FILLER inside the attention j-loop of head
    pair qt-1, so the PE never waits on the score->exp->affine->AV chain.
    Scores land in <=512-col psum pieces (1 bank each) to fit the psum
    budget: 3 score pieces + 4 opsums + 1 proj = 8 banks.
  Pair 7's attention j-loop uses the first output-projection (stage E)
    chain as filler; E accumulates each (ot,sc) psum over the 8 o-tiles
    with the last-finishing tile placed last in the chain.
"""

import numpy as np
import ml_dtypes

B, S, DIM = 4, 1024, 2048
NH, NKV, HD = 32, 8, 64
P = 128
ND = DIM // P  # 16 d-tiles

_SWAP_ADJ = [i ^ 1 for i in range(32)]  # pairwise partition swap within quadrants

_CACHE = {}


def host_prep(x, freqs_cos, freqs_sin, wqkv, wo):
    """Build the 8 per-core input dicts (bf16 weights/activations)."""
    bf16 = ml_dtypes.bfloat16
    x = np.asarray(x, np.float32)
    wqkv = np.asarray(wqkv, np.float32)
    wo = np.asarray(wo, np.float32)
    cos = np.asarray(freqs_cos, np.float32)
    sin = np.asarray(freqs_sin, np.float32)

    cosT, sinT = cos.T, sin.T                      # [32, S]
    C64 = np.repeat(cosT, 2, axis=0)               # [64, S]
    Ss64 = np.repeat(sinT, 2, axis=0).copy()
    Ss64[0::2] *= -1.0                             # even rows -sin, odd +sin
    C64 = np.ascontiguousarray(C64, dtype=np.float32)
    Ss64 = np.ascontiguousarray(Ss64, dtype=np.float32)
    scale = np.float32(1.0 / np.sqrt(HD))
    Cq, Sq = C64 * scale, Ss64 * scale      # [64, S]; kernel duplicates rows
    Ck, Sk = C64, Ss64

    woT_full = np.ascontiguousarray(wo.T.astype(bf16))          # [d', o]
    xT_full = np.ascontiguousarray(
        x.transpose(0, 2, 1).astype(bf16))  # [B, DIM, S]
    wqkvT_full = np.ascontiguousarray(wqkv.T.astype(bf16))      # [DIM, 3072]
    in_maps = []
    for core in range(8):
        b, hh = core // 2, core % 2
        groups = range(4 * hh, 4 * hh + 4)
        qheads = range(16 * hh, 16 * hh + 16)
        # column order: k groups (256) | v groups (256) | q heads (1024)
        wqkvT = np.empty((DIM, 1536), bf16)
        col = 0
        blocks = ([(g * 6 + 4) * 64 for g in groups]
                  + [(g * 6 + 5) * 64 for g in groups]
                  + [(h // 4 * 6 + h % 4) * 64 for h in qheads])
        for c0 in blocks:
            wqkvT[:, col:col + 64] = wqkvT_full[:, c0:c0 + 64]
            col += 64
        in_maps.append({
            "xT": xT_full[b],                                      # [2048, 1024]
            "wqkvT": wqkvT,                                        # [2048, 1536]
            "woT": np.ascontiguousarray(woT_full[1024 * hh:1024 * hh + 1024]),
            "Cq": Cq.astype(bf16), "Sq": Sq.astype(bf16),
            "Ck": Ck.astype(bf16), "Sk": Sk.astype(bf16),
        })
    return in_maps


def build_nc(reps=1):
    from contextlib import ExitStack
    import concourse.bacc as bacc
    import concourse.bass as bass
    import concourse.tile as tile
    import concourse.mybir as mybir

    f32 = mybir.dt.float32
    bf16 = mybir.dt.bfloat16
    EXP = mybir.ActivationFunctionType.Exp

    nc = bacc.Bacc("TRN2", target_bir_lowering=False, debug=False)
    xT_d = nc.dram_tensor("xT", [DIM, S], bf16, kind="ExternalInput")
    wqkvT_d = nc.dram_tensor("wqkvT", [DIM, 1536], bf16, kind="ExternalInput")
    woT_d = nc.dram_tensor("woT", [1024, DIM], bf16, kind="ExternalInput")
    Cq_d = nc.dram_tensor("Cq", [64, S], bf16, kind="ExternalInput")
    Sq_d = nc.dram_tensor("Sq", [64, S], bf16, kind="ExternalInput")
    Ck_d = nc.dram_tensor("Ck", [64, S], bf16, kind="ExternalInput")
    Sk_d = nc.dram_tensor("Sk", [64, S], bf16, kind="ExternalInput")
    out_d = nc.dram_tensor("out", [S, DIM], bf16, kind="ExternalOutput")

    def emit(tc, pfx):
        with ExitStack() as stack:
            resid = stack.enter_context(tc.tile_pool(name=pfx + "resid", bufs=1))

            def rtile(shape, dt_, nm):
                return resid.tile(shape, dt_, tag=pfx + nm, name=pfx + nm)

            # q/k/o tiles split into 512-col halves: tile-granular dependency
            # tracking then lets consumers of one half start without waiting
            # for the writer of the other half.
            q_sb = {(i, t): rtile([P, 512], bf16, f"q{i}_{t}")
                    for i in range(8) for t in (0, 1)}
            k_sb = {(g, t): rtile([P, 512], bf16, f"k{g}_{t}")
                    for g in range(4) for t in (0, 1)}
            vaug = [rtile([P, 4, 65], bf16, f"va{i}") for i in range(8)]
            o_sb = {(i, t): rtile([P, 512], bf16, f"o{i}_{t}")
                    for i in range(8) for t in (0, 1)}

            xres_pool = stack.enter_context(
                tc.tile_pool(name=pfx + "xres", bufs=1))
            rc_pool = stack.enter_context(
                tc.tile_pool(name=pfx + "ropeconst", bufs=1))
            rt_pool = stack.enter_context(
                tc.tile_pool(name=pfx + "ropetmp", bufs=2))

            # ---------------- DMA phase 0 ----------------
            xres, wkv = [], []
            c_sb = {}
            wq_pool = stack.enter_context(tc.tile_pool(name=pfx + "wq", bufs=2))
            p1stack = ExitStack()  # phase-1-scoped pools (freed before phase 2)
            w_pool = p1stack.enter_context(tc.tile_pool(name=pfx + "wkv", bufs=1))
            for d in range(ND):
                xt = xres_pool.tile([P, S], bf16, tag=f"x{d}", name=pfx + f"x{d}")
                wt = w_pool.tile([P, 512], bf16, tag=f"wkv{d}",
                                 name=pfx + f"wkv{d}")
                eng_a = nc.sync if d % 2 == 0 else nc.scalar
                eng_b = nc.scalar if d % 2 == 0 else nc.sync
                eng_a.dma_start(out=xt[:, 0:512],
                                in_=xT_d[d * P:(d + 1) * P, 0:512])
                eng_b.dma_start(out=wt[:],
                                in_=wqkvT_d[d * P:(d + 1) * P, 0:512])
                xres.append(xt)
                wkv.append(wt)
            for d in range(ND):
                (nc.sync if d % 2 == 0 else nc.scalar).dma_start(
                    out=xres[d][:, 512:S],
                    in_=xT_d[d * P:(d + 1) * P, 512:S])
                if d == 7:  # rope tables: DMA 64 rows, duplicate on ACT
                    for i, (nm, dr) in enumerate(
                            (("Cq", Cq_d), ("Sq", Sq_d),
                             ("Ck", Ck_d), ("Sk", Sk_d))):
                        ct = rc_pool.tile([P, S], bf16, tag=nm, name=pfx + nm)
                        (nc.sync if i % 2 else nc.scalar).dma_start(
                            out=ct[0:64, :], in_=dr[:])
                        nc.scalar.copy(ct[64:128, :], ct[0:64, :])
                        c_sb[nm] = ct

            # q weight stream: issue the group-0 DMAs right behind phase-0
            wq_tiles = {}

            def load_wq_group(qg):
                for d in range(ND):
                    wt = wq_pool.tile([P, 512], bf16, tag=f"wq{d}",
                                      name=pfx + f"wq{qg}_{d}")
                    (nc.sync if d % 2 else nc.scalar).dma_start(
                        out=wt[:],
                        in_=wqkvT_d[d * P:(d + 1) * P,
                                    512 + qg * 512:1024 + qg * 512])
                    wq_tiles[(qg, d)] = wt

            load_wq_group(0)

            # ---------------- Phase 1: k proj + rope, v proj ----------------
            if True:
                psK = p1stack.enter_context(
                    tc.tile_pool(name=pfx + "psK", bufs=4, space="PSUM"))
                psV = p1stack.enter_context(
                    tc.tile_pool(name=pfx + "psV", bufs=3, space="PSUM"))

                # PE warmup: spin matmuls on const data while first DMAs land
                wmt = rt_pool.tile([P, P], bf16, tag="sh", name=pfx + "wm")
                nc.vector.tensor_copy(wmt[:], nc.const_aps.tensor(0.0, (P, P), f32))
                wps = psV.tile([P, 256], f32, tag="vp", name=pfx + "wps")
                for _ in range(16):
                    nc.tensor.matmul(wps[:, 0:128], wmt[:], wmt[:],
                                     start=True, stop=True)

                # k projection: 4 chains (at,t), d-outer so it starts early
                kps = {}
                for at in (0, 1):
                    for t in (0, 1):
                        kps[(at, t)] = psK.tile([P, 512], f32, tag="kp",
                                                name=pfx + f"kp{at}_{t}")
                for d in range(ND):
                    for at in (0, 1):
                        for t in (0, 1):
                            nc.tensor.matmul(
                                kps[(at, t)][:],
                                wkv[d][:, at * P:(at + 1) * P],
                                xres[d][:, t * 512:(t + 1) * 512],
                                start=(d == 0), stop=(d == ND - 1))

                rope_ctr = [0]

                def rope(ptile, is_q, sl, dst):
                    """dst[:, :] = rope(ptile); sl selects the table columns."""
                    C_ = c_sb["Cq" if is_q else "Ck"]
                    S_ = c_sb["Sq" if is_q else "Sk"]
                    rope_ctr[0] += 1
                    i = rope_ctr[0]
                    sh = rt_pool.tile([P, 512], f32, tag="sh",
                                      name=pfx + f"sh{i}")
                    m1 = rt_pool.tile([P, 512], f32, tag="m1",
                                      name=pfx + f"m1_{i}")
                    m2 = rt_pool.tile([P, 512], f32, tag="m2",
                                      name=pfx + f"m2_{i}")
                    nc.vector.stream_shuffle(sh[:], ptile[:], _SWAP_ADJ)
                    nc.vector.tensor_mul(m1[:], ptile[:], C_[:, sl])
                    nc.gpsimd.tensor_mul(m2[:], sh[:], S_[:, sl])
                    nc.gpsimd.tensor_add(dst[:], m1[:], m2[:])

                # rope k -> k_sb (dup halves so both par-halves see the group)
                ro_pool = p1stack.enter_context(
                    tc.tile_pool(name=pfx + "ro", bufs=2))
                for at in (0, 1):
                    for t in (0, 1):
                        sl = slice(t * 512, t * 512 + 512)
                        ro = ro_pool.tile([P, 512], bf16, tag="ro",
                                          name=pfx + f"ro{at}_{t}")
                        C_ = c_sb["Ck"]
                        S_ = c_sb["Sk"]
                        sh = rt_pool.tile([P, 512], f32, tag="sh",
                                          name=pfx + f"ksh{at}_{t}")
                        m1 = rt_pool.tile([P, 512], f32, tag="m1",
                                          name=pfx + f"km1_{at}_{t}")
                        m2 = rt_pool.tile([P, 512], f32, tag="m2",
                                          name=pfx + f"km2_{at}_{t}")
                        nc.vector.stream_shuffle(sh[:], kps[(at, t)][:], _SWAP_ADJ)
                        nc.vector.tensor_mul(m1[:], kps[(at, t)][:], C_[:, sl])
                        nc.gpsimd.tensor_mul(m2[:], sh[:], S_[:, sl])
                        nc.gpsimd.tensor_add(ro[:], m1[:], m2[:])
                        for half in (0, 1):
                            g = 2 * at + half
                            src = ro[half * 64:half * 64 + 64, :]
                            nc.scalar.copy(k_sb[(g, t)][0:64, :], src)
                            nc.vector.tensor_copy(k_sb[(g, t)][64:128, :], src)

                # v projection (natural layout) + ones augmentation
                for st in range(8):
                    pt = psV.tile([P, 256], f32, tag="vp", name=pfx + f"vp{st}")
                    for d in range(ND):
                        nc.tensor.matmul(
                            pt[:], xres[d][:, st * P:(st + 1) * P],
                            wkv[d][:, 256:512], start=(d == 0), stop=(d == ND - 1))
                    nc.vector.tensor_copy(vaug[st][:, :, 64],
                                          nc.const_aps.tensor(1.0, (P, 4), f32))
                    for g in range(4):
                        if g % 2:
                            nc.scalar.copy(
                                vaug[st][:, g, 0:64], pt[:, g * 64:(g + 1) * 64])
                        else:
                            nc.vector.tensor_copy(
                                vaug[st][:, g, 0:64], pt[:, g * 64:(g + 1) * 64])

            p1stack.close()

            # ---------------- Phase 2: q proj blocks + attention ----------------
            wo_pool = stack.enter_context(tc.tile_pool(name=pfx + "wo", bufs=1))
            wo_sb = {}

            def load_wo(i):
                ot, dt_ = i // 8, i % 8
                w = wo_pool.tile([P, 512], bf16, tag=f"wo{ot}_{dt_}",
                                 name=pfx + f"wo{ot}_{dt_}")
                nc.scalar.dma_start(
                    out=w[:],
                    in_=woT_d[dt_ * P:(dt_ + 1) * P, ot * 512:(ot + 1) * 512])
                wo_sb[(ot, dt_)] = w

            e_pool = stack.enter_context(tc.tile_pool(name=pfx + "expT", bufs=12))
            rd_pool = stack.enter_context(
                tc.tile_pool(name=pfx + "rdram", bufs=4, space="DRAM"))
            n_pool = stack.enter_context(tc.tile_pool(name=pfx + "normtmp", bufs=4))
            ob_pool = stack.enter_context(tc.tile_pool(name=pfx + "outsb", bufs=3))
            psP = stack.enter_context(
                tc.tile_pool(name=pfx + "psumP", bufs=2, space="PSUM"))
            psOp = stack.enter_context(
                tc.tile_pool(name=pfx + "psumO", bufs=4, space="PSUM"))
            psS = stack.enter_context(
                tc.tile_pool(name=pfx + "psumS", bufs=2, space="PSUM"))

            def normalize(h, t, opsum):
                par = h % 2
                r = n_pool.tile([1, 512], f32, tag="r", name=pfx + f"r{h}_{t}")
                nc.vector.reciprocal(r[:], opsum[64:65, :])
                rd = rd_pool.tile([1, 512], f32, tag="rd", name=pfx + f"rd{h}_{t}")
                nc.sync.dma_start(out=rd[:], in_=r[:])
                rb = n_pool.tile([64, 512], f32, tag="rb", name=pfx + f"rb{h}_{t}")
                rdap = rd[:]
                bcast = bass.AP(tensor=rdap.tensor, offset=rdap.offset,
                                ap=[[0, 64]] + [list(p) for p in rdap.ap[1:]])
                nc.sync.dma_start(out=rb[:], in_=bcast)
                dst = o_sb[(h // 2, t)][par * 64:par * 64 + 64, :]
                nc.vector.tensor_mul(dst, opsum[0:64, :], rb[:])

            # --- attention pair machinery (generator-style interleave) ---
            def head_pair_steps(p):
                """Yield per-j emission closures for head pair (2p, 2p+1).

                Each yielded item is (pre_fn, post_fn): pre emits scores+exp
                +affine for slot j, post emits the AV matmuls. The proj
                filler matmuls go between pre and post.
                """
                heads = (2 * p, 2 * p + 1)
                g = p // 2
                opsums = {}
                for h in heads:
                    opsums[(h, 0)] = psOp.tile(
                        [65, 512], f32, tag="op", name=pfx + f"op{h}_0")
                ets = {}

                def pre(j):
                    lo = j * P
                    for h in heads:
                        par = h % 2
                        sl64 = slice(par * 64, par * 64 + 64)
                        qa = q_sb[(h // 2, 0)][sl64, :]
                        qb = q_sb[(h // 2, 1)][sl64, :]
                        et = e_pool.tile([P, S], bf16, tag="et",
                                         name=pfx + f"et{h}_{j}")
                        ets[(h, j)] = et
                        if j < 4:
                            kh = k_sb[(g, 0)][sl64, :]
                            pa = psS.tile([P, 512 - lo], f32, tag="sp",
                                          name=pfx + f"spa{h}_{j}")
                            nc.tensor.matmul(pa[:], kh[:, lo:lo + P],
                                             qa[:, lo:512], start=True, stop=True)
                            pb = psS.tile([P, 512], f32, tag="sp",
                                          name=pfx + f"spb{h}_{j}")
                            nc.tensor.matmul(pb[:], kh[:, lo:lo + P],
                                             qb[:], start=True, stop=True)
                            nc.scalar.activation(et[:, lo:512], pa[:], EXP)
                            nc.scalar.activation(et[:, 512:S], pb[:], EXP)
                        else:
                            kh = k_sb[(g, 1)][sl64, :]
                            pb = psS.tile([P, S - lo], f32, tag="sp",
                                          name=pfx + f"spb{h}_{j}")
                            nc.tensor.matmul(pb[:], kh[:, lo - 512:lo - 512 + P],
                                             qb[:, lo - 512:512],
                                             start=True, stop=True)
                            nc.scalar.activation(et[:, lo:S], pb[:], EXP)
                        # diagonal chunk: zero sq < sk
                        nc.gpsimd.affine_select(
                            out=et[:, lo:lo + P], in_=et[:, lo:lo + P],
                            pattern=[[1, P]], channel_multiplier=-1,
                            base=0, compare_op=mybir.AluOpType.is_ge, fill=0.0)

                def post(j):
                    # t0 opsum accumulates j 0..3; the t1-opsum contributions
                    # of j<4 are DEFERRED to slot j==4 so the t1 psum banks
                    # are allocated mid-block (after the previous pair's t1
                    # normalize has released its banks) -- no PE stall.
                    lo = j * P
                    for h in heads:
                        if j < 4:
                            et = ets[(h, j)]
                            nc.tensor.matmul(
                                opsums[(h, 0)][0:65, lo:512], vaug[j][:, g, :],
                                et[:, lo:512], start=(j == 0), stop=(j == 3))
                            if j == 3:
                                normalize(h, 0, opsums[(h, 0)])
                        else:
                            if j == 4:
                                opsums[(h, 1)] = psOp.tile(
                                    [65, 512], f32, tag="op",
                                    name=pfx + f"op{h}_1")
                                for jj in range(4):
                                    et = ets.pop((h, jj))
                                    nc.tensor.matmul(
                                        opsums[(h, 1)][0:65, 0:512],
                                        vaug[jj][:, g, :], et[:, 512:S],
                                        start=(jj == 0), stop=False)
                            lo1 = lo - 512
                            et = ets.pop((h, j))
                            nc.tensor.matmul(
                                opsums[(h, 1)][0:65, lo1:512], vaug[j][:, g, :],
                                et[:, 512 + lo1:S], start=False, stop=(j == 7))

                def finish():
                    for h in heads:
                        normalize(h, 1, opsums[(h, 1)])

                return pre, post, finish

            def proj_chunks(qt):
                """Yield closures each emitting 4 proj matmuls for q-tile qt."""
                qg, a2 = qt // 4, qt % 4
                pts = {}

                def chunk(i):
                    # i in 0..7; t = i//4, d-range = (i%4)*4..+4
                    t = i // 4
                    if i % 4 == 0:
                        pool = psS if (qt == 0 and t == 1) else psP
                        pts[t] = pool.tile([P, 512], f32,
                                           tag="sp" if pool is psS else "pp",
                                           name=pfx + f"pp{qt}_{t}")
                    for d in range((i % 4) * 4, (i % 4) * 4 + 4):
                        nc.tensor.matmul(
                            pts[t][:],
                            wq_tiles[(qg, d)][:, a2 * P:(a2 + 1) * P],
                            xres[d][:, t * 512:(t + 1) * 512],
                            start=(d == 0), stop=(d == ND - 1))
                    if i % 4 == 3:
                        # chain done: rope into q_sb[(qt, t)]
                        sl = slice(t * 512, (t + 1) * 512)
                        rope(pts[t], True, sl, q_sb[(qt, t)])

                return chunk

            # blocks: block 0 = proj(0) alone; block b>=1 = proj(b) + C(b-1).
            # pre() is emitted ONE slot ahead of post() (and the next pair's
            # pre(0) at block end) so the boundary AV never waits on exp.
            prev = None
            for qt in range(8):
                if qt == 2:
                    load_wq_group(1)
                chunk = proj_chunks(qt)
                nxt = head_pair_steps(qt)
                if prev is None:
                    for i in range(8):
                        chunk(i)
                else:
                    pre, post, finish = prev
                    for j in range(8):
                        if j < 7:
                            pre(j + 1)
                        chunk(j)
                        post(j)
                    nxt[0](0)
                    finish()
                if prev is None:
                    nxt[0](0)
                prev = nxt
                for i in range(4):
                    load_wo(4 * qt + i)

            # pair 7: E-chain filler
            pre, post, finish = prev
            e_chains = []  # sc-major: chains reading the t0 o-halves first
            for sc in range(8):
                for ot in range(4):
                    e_chains.append((ot, sc))

            def e_chain(idx, pool, upto):
                ot, sc = e_chains[idx]
                pe = pool.tile([P, 512], f32, tag="op" if pool is psOp else
                               ("pp" if pool is psP else "sp"),
                               name=pfx + f"pe{ot}_{sc}")
                for dt_ in range(upto):
                    nc.tensor.matmul(
                        pe[:], o_sb[(dt_, sc // 4)][:, (sc % 4) * P:(sc % 4 + 1) * P],
                        wo_sb[(ot, dt_)][:],
                        start=(dt_ == 0), stop=(dt_ == 7))
                return pe

            def e_finish(pe, idx):
                ot, sc = e_chains[idx]
                nc.tensor.matmul(
                    pe[:], o_sb[(7, sc // 4)][:, (sc % 4) * P:(sc % 4 + 1) * P],
                    wo_sb[(ot, 7)][:], start=False, stop=True)
                ob = ob_pool.tile([P, 512], bf16, tag="ob",
                                  name=pfx + f"ob{ot}_{sc}")
                nc.vector.tensor_copy(ob[:], pe[:])
                (nc.sync if sc % 2 else nc.scalar).dma_start(
                    out=out_d[sc * P:(sc + 1) * P, ot * 512:(ot + 1) * 512],
                    in_=ob[:])

            # chain 0 rides psP as filler for pair 7's j-loop
            pe0 = None
            for j in range(8):
                if j < 7:
                    pre(j + 1)
                if j == 0:
                    pe0 = e_chain(0, psP, 7)  # dt 0..6 while pair 7 cooks
                post(j)
            finish()
            e_finish(pe0, 0)
            for idx in range(1, 32):
                pool = psS if idx % 3 != 0 else psOp
                pe = e_chain(idx, pool, 7)
                e_finish(pe, idx)

    with tile.TileContext(nc) as tc:
        for rep in range(reps):
            emit(tc, f"r{rep}_" if reps > 1 else "")

    nc.compile()
    return nc


def _get_nc():
    if "nc" not in _CACHE:
        _CACHE["nc"] = build_nc()
    return _CACHE["nc"]


def kernel(**inputs):
    from concourse.bass_utils import run_bass_kernel_spmd
    nc = _get_nc()
    in_maps = host_prep(**inputs)
    res = run_bass_kernel_spmd(nc, in_maps, core_ids=list(range(8)))
    outs = [np.asarray(res.results[c]["out"], np.float32) for c in range(8)]
    full = np.stack([outs[2 * b] + outs[2 * b + 1] for b in range(B)])
    return full.astype(np.float32)


if __name__ == "__main__":
    nc = build_nc()
    print("build ok")
